# revision 29
# baseline (speedup 1.0000x reference)
"""Trainium2 Bass kernel for nn_MultiHeadAttention_89524298317897 (v9).

Data-parallel over batch: core b computes batch element b end-to-end.
All on-device tensors bf16 (host pre-casts + pre-transposes); PSUM
accumulation fp32.

Math per core (batch b), faithful to torch's .view head split (chunks
the sequence dim): head h token t <-> qp[64h + t//16, (t%16)*64 + d].
Key/query token order inside the kernel is the fixed permutation
m = t%16 = 2c + j, row = t//16  ->  free index f = j*512 + c*64 + row;
attention is permutation invariant and the output eviction un-permutes.

Layouts (per partition p):
  qS/kS [128, 16, 1024]  xS[64*jj + d, h, f] = xpT[(2c+j)*64+d, 64h+row]
        (qS only needs the diagonal halves; kS is dup'd via 2 swap DMAs)
  vS    [128, 16, 8, 64] vS[j*64 + krow, h, c, d] = vp[64h+krow, (2c+j)*64+d]
  ET    [128, 8, 2, 512] exp(scores) per head: [key-in-chunk, c, pi, q]
  attn  [128, 8, 1024]   out^T: attn[o%128, o//128, s]

Per head h: scores = 4 quadrant MMs per chunk (row=pi x col=j, K=64,
M=64, N=512) -> exp (ACT, N=1024 ops) -> PV: 2 col-streams x 8
accumulating K=128 MMs -> Z via ones lhsT -> DVE reciprocal + multiply
eviction. The head loop software-pipelines at chunk granularity: the
in-order PE queue gets dense ungated work (PV/Z of the previous head)
before each exp-gated scores matmul, with v-projection and
output-projection tiles interleaved between heads so the PE never
sits idle while ACT computes exp (keeps the HAM clock warm).
"""
import os
import sys

for _p in ("/opt/trn_rl_repo",):
    if os.path.isdir(_p) and _p not in sys.path:
        sys.path.insert(0, _p)

import numpy as np
import ml_dtypes
import concourse.bass as bass
import concourse.mybir as mybir
import concourse.tile as tile
from concourse import bacc
from concourse.bass_utils import run_bass_kernel_spmd

B, S, D, NH, DH = 8, 1024, 1024, 16, 64
P = 128
F32 = mybir.dt.float32
BF16 = mybir.dt.bfloat16
EXP_FN = mybir.ActivationFunctionType.Exp

_CACHE: dict = {}


def _build_nc():
    nc = bacc.Bacc("TRN2", target_bir_lowering=False, debug=False)

    qT = nc.dram_tensor("qT", [D, S], BF16, kind="ExternalInput")
    kT = nc.dram_tensor("kT", [D, S], BF16, kind="ExternalInput")
    vT = nc.dram_tensor("vT", [D, S], BF16, kind="ExternalInput")
    wqT = nc.dram_tensor("wqT", [D, D], BF16, kind="ExternalInput")
    wkT = nc.dram_tensor("wkT", [D, D], BF16, kind="ExternalInput")
    wvT = nc.dram_tensor("wvT", [D, D], BF16, kind="ExternalInput")
    woT = nc.dram_tensor("woT", [D, D], BF16, kind="ExternalInput")
    out = nc.dram_tensor("out", [S, D], F32, kind="ExternalOutput")

    def part3(dram):  # [1024, X] -> [128, 8, X] with row = io*128 + p
        return dram[:].rearrange("(io p) x -> p io x", p=P)

    with tile.TileContext(nc) as tc:
        with tc.tile_pool(name="big", bufs=1) as big, \
             tc.tile_pool(name="pa_x", bufs=2) as pa_x, \
             tc.tile_pool(name="pa_w", bufs=2) as pa_w, \
             tc.tile_pool(name="pet", bufs=2) as pet, \
             tc.tile_pool(name="psb", bufs=2) as psb, \
             tc.tile_pool(name="pconst", bufs=1) as pconst, \
             tc.tile_pool(name="psc", bufs=2, space="PSUM") as psc, \
             tc.tile_pool(name="pnz", bufs=2, space="PSUM") as pnz:

            qS = big.tile([P, NH, S], BF16)
            kS = big.tile([P, NH, S], BF16)
            vS = big.tile([P, NH, 8, DH], BF16)
            attn = big.tile([P, 8, S], BF16)
            out3 = out[:].rearrange("(sc p) o -> p sc o", p=P)

            ones_bf = pconst.tile([P, 64], BF16)
            nc.gpsimd.memset(ones_bf[:], 1.0)
            # warm-up burst: lift the HAM clock gate to 8/8 before the
            # first real matmuls (which are gated on input DMA anyway)
            warm = pconst.tile([P, 128], BF16)
            nc.gpsimd.memset(warm[:], 0.0)
            wps = psc.tile([P, 1024], F32, tag="sc", name="wps")
            for i in range(100):
                nc.tensor.matmul(wps[:, 0:128], warm[:], warm[:],
                                 start=True, stop=True, skip_group_check=True)

            # ---------- input DMAs (sync queue) ----------
            xq = pa_x.tile([P, 8, S], BF16, tag="x")
            wq = pa_w.tile([P, 8, D], BF16, tag="w")
            for io in range(8):  # chunked so first matmuls start early
                nc.sync.dma_start(wq[:, io, :], part3(wqT)[:, io, :])
                nc.sync.dma_start(xq[:, io, :], part3(qT)[:, io, :])
            xk = pa_x.tile([P, 8, S], BF16, tag="x")
            wk = pa_w.tile([P, 8, D], BF16, tag="w")
            for io in range(8):
                nc.sync.dma_start(wk[:, io, :], part3(wkT)[:, io, :])
                nc.sync.dma_start(xk[:, io, :], part3(kT)[:, io, :])

            # ---------- phase A: q/k projections (transposed out) ----------
            def evict_qk(dst, ps_half, mt, nchunk):
                hs0 = nchunk * 8
                nc.vector.tensor_copy(
                    dst[0:64, hs0:hs0 + 8, mt * 64:(mt + 1) * 64],
                    ps_half[0:64].rearrange("p (a b) -> p a b", a=8))
                nc.vector.tensor_copy(
                    dst[64:128, hs0:hs0 + 8, 512 + mt * 64:512 + (mt + 1) * 64],
                    ps_half[64:128].rearrange("p (a b) -> p a b", a=8))

            for nchunk in range(2):
                for xt, wt, dst in ((xq, wq, qS), (xk, wk, kS)):
                    for jp in range(2):  # pair of j-tiles, io-interleaved
                        tiles = []
                        for j in (2 * jp, 2 * jp + 1):
                            ps = psc.tile([P, 1024], F32, tag="sc",
                                          name=f"ps{j}")
                            tiles.append((j, ps))
                        for io in range(8):
                            for j, ps in tiles:
                                for f in range(2):
                                    mt = 2 * j + f
                                    # col-split M=64 chains share one rhs
                                    for ch in range(2):
                                        nc.tensor.matmul(
                                            ps[ch * 64:(ch + 1) * 64,
                                               f * 512:(f + 1) * 512],
                                            wt[:, io, mt * P + ch * 64:
                                               mt * P + (ch + 1) * 64],
                                            xt[:, io,
                                               nchunk * 512:(nchunk + 1) * 512],
                                            start=(io == 0), stop=(io == 7),
                                            skip_group_check=True)
                        for j, ps in tiles:
                            for f in range(2):
                                evict_qk(dst, ps[:, f * 512:(f + 1) * 512],
                                         2 * j + f, nchunk)
                # kS needs both partition-half duplicates
                hs = slice(nchunk * 8, nchunk * 8 + 8)
                nc.gpsimd.dma_start(kS[64:128, hs, 0:512], kS[0:64, hs, 0:512])
                nc.gpsimd.dma_start(kS[0:64, hs, 512:1024],
                                    kS[64:128, hs, 512:1024])

            # ---------- v / wo loads ----------
            xv = pa_x.tile([P, 8, S], BF16, tag="x")
            wv = pa_w.tile([P, 8, D], BF16, tag="w")
            nc.sync.dma_start(wv[:], part3(wvT))
            nc.sync.dma_start(xv[:], part3(vT))
            wo = pa_w.tile([P, 8, D], BF16, tag="w")
            nc.sync.dma_start(wo[:], part3(woT))

            # ---------- helpers emitted inside the head loop ----------
            def evict_vproj(halves, mtv):
                for f in range(2):
                    sc2 = psb.tile([P, 4, 64], BF16, tag="scratch")
                    psn = halves[f]
                    lo = psn[0:64].rearrange("p (a e d) -> p a e d", a=4, e=2)
                    hi = psn[64:128].rearrange("p (a e d) -> p a e d", a=4, e=2)
                    cs = slice(4 * f, 4 * f + 4)
                    nc.vector.tensor_copy(vS[0:64, 2 * mtv, cs, :],
                                          lo[:, :, 0, :])
                    nc.vector.tensor_copy(sc2[0:64], lo[:, :, 1, :])
                    nc.vector.tensor_copy(sc2[64:128], hi[:, :, 0, :])
                    nc.vector.tensor_copy(vS[64:128, 2 * mtv + 1, cs, :],
                                          hi[:, :, 1, :])
                    nc.gpsimd.dma_start(vS[64:128, 2 * mtv, cs, :], sc2[0:64])
                    nc.gpsimd.dma_start(vS[0:64, 2 * mtv + 1, cs, :],
                                        sc2[64:128])

            def emit_vproj(mtv):
                ps = psc.tile([P, 1024], F32, tag="sc")
                for f in range(2):  # nchunk
                    for io in range(8):
                        nc.tensor.matmul(
                            ps[:, f * 512:(f + 1) * 512],
                            xv[:, io, mtv * P:(mtv + 1) * P],
                            wv[:, io, f * 512:(f + 1) * 512],
                            start=(io == 0), stop=(io == 7))
                evict_vproj([ps[:, 0:512], ps[:, 512:1024]], mtv)

            def emit_phaseC(st):
                ps = psc.tile([P, 1024], F32, tag="sc")
                for oc in range(2):
                    for jc in range(8):
                        nc.tensor.matmul(
                            ps[:, oc * 512:(oc + 1) * 512],
                            attn[:, jc, st * P:(st + 1) * P],
                            wo[:, jc, oc * 512:(oc + 1) * 512],
                            start=(jc == 0), stop=(jc == 7))
                res = psb.tile([P, 1024], F32, tag="res")
                nc.vector.tensor_copy(res[:], ps[:])
                nc.gpsimd.dma_start(out3[:, st, :], res[:])

            # ---------- phase B: software-pipelined head loop ----------
            et_tiles = {}
            for h in range(NH + 1):
                ET = None
                if h < NH:
                    ET = pet.tile([P, 8, 2, 512], BF16, tag="et")
                    et_tiles[h] = ET
                hp = h - 1
                ETp = et_tiles.pop(hp) if hp >= 0 else None
                v0 = None
                if h == 0:
                    v0a = pnz.tile([P, 512], F32, tag="num", name="v0a")
                    v0b = pnz.tile([P, 512], F32, tag="z", name="v0b")
                    v0 = [v0a, v0b]
                elif h < NH and h % 2 == 0:
                    emit_vproj(h // 2)
                    if h >= 4:
                        emit_phaseC((h - 4) // 2)
                if ETp is not None:
                    num = pnz.tile([P, 512], F32, tag="num")
                    z = pnz.tile([P, 512], F32, tag="z")
                for r in range(8):
                    if v0 is not None:  # head-0 pipeline fill: vproj(0)
                        for f in range(2):
                            nc.tensor.matmul(
                                v0[f][:], xv[:, r, 0:P],
                                wv[:, r, f * 512:(f + 1) * 512],
                                start=(r == 0), stop=(r == 7),
                                skip_group_check=True)
                    if ETp is not None:
                        for pi in range(2):
                            nc.tensor.matmul(
                                num[64 * pi:64 * pi + 64, :],
                                vS[:, hp, r, :], ETp[:, r, pi, :],
                                start=(r == 0), stop=(r == 7),
                                skip_group_check=True)
                        for pi in range(2):
                            nc.tensor.matmul(
                                z[64 * pi:64 * pi + 64, :],
                                ones_bf[:], ETp[:, r, pi, :],
                                start=(r == 0), stop=(r == 7),
                                skip_group_check=True)
                    if ET is not None:
                        ps = psc.tile([P, 1024], F32, tag="sc")
                        for pi in range(2):
                            row = slice(64 * pi, 64 * pi + 64)
                            rhs = qS[row, h, pi * 512:(pi + 1) * 512]
                            for j in range(2):
                                lhsT = kS[row, h,
                                          j * 512 + r * 64:j * 512 + (r + 1) * 64]
                                nc.tensor.matmul(
                                    ps[64 * j:64 * j + 64,
                                       pi * 512:(pi + 1) * 512], lhsT, rhs,
                                    start=True, stop=True,
                                    skip_group_check=True)
                        nc.scalar.activation(
                            ET[:, r, :, :],
                            ps[:].rearrange("p (a b) -> p a b", a=2), EXP_FN)
                if v0 is not None:
                    evict_vproj([v0[0][:], v0[1][:]], 0)
                if ETp is not None:
                    zr = psb.tile([P, 512], F32, tag="zr", bufs=1)
                    nc.vector.reciprocal_approx_fast(zr[:], z[:])
                    nc.vector.tensor_tensor(
                        attn[:, 0:8, 64 * hp:64 * hp + 64],
                        num[:].rearrange("p (a b) -> p a b", a=8),
                        zr[:].rearrange("p (a b) -> p a b", a=8),
                        mybir.AluOpType.mult)
            emit_phaseC(6)
            emit_phaseC(7)

    nc.compile()
    return nc


def _get_nc():
    if "nc" not in _CACHE:
        _CACHE["nc"] = _build_nc()
    return _CACHE["nc"]


def _prep_inputs(q, k, v, w_q, w_k, w_v, w_o):
    bf = ml_dtypes.bfloat16
    wqT = np.ascontiguousarray((np.asarray(w_q, np.float32) / 8.0).T).astype(bf)
    wkT = np.ascontiguousarray(np.asarray(w_k, np.float32).T).astype(bf)
    wvT = np.ascontiguousarray(np.asarray(w_v, np.float32).T).astype(bf)
    woT = np.ascontiguousarray(np.asarray(w_o, np.float32).T).astype(bf)
    in_maps = []
    for b in range(B):
        in_maps.append({
            "qT": np.ascontiguousarray(np.asarray(q[b], np.float32).T).astype(bf),
            "kT": np.ascontiguousarray(np.asarray(k[b], np.float32).T).astype(bf),
            "vT": np.ascontiguousarray(np.asarray(v[b], np.float32).T).astype(bf),
            "wqT": wqT, "wkT": wkT, "wvT": wvT, "woT": woT,
        })
    return in_maps


def kernel(q, k, v, mask, w_q, w_k, w_v, w_o, **_ignored):
    nc = _get_nc()
    in_maps = _prep_inputs(q, k, v, w_q, w_k, w_v, w_o)
    res = run_bass_kernel_spmd(nc, in_maps, core_ids=list(range(B)))
    return np.stack([res.results[b]["out"] for b in range(B)]).astype(np.float32)


# revision 30
# speedup vs baseline: 1.1201x; 1.1201x over previous
"""Trainium2 Bass kernel for nn_MultiHeadAttention_89524298317897 (v9).

Data-parallel over batch: core b computes batch element b end-to-end.
All on-device tensors bf16 (host pre-casts + pre-transposes); PSUM
accumulation fp32.

Math per core (batch b), faithful to torch's .view head split (chunks
the sequence dim): head h token t <-> qp[64h + t//16, (t%16)*64 + d].
Key/query token order inside the kernel is the fixed permutation
m = t%16 = 2c + j, row = t//16  ->  free index f = j*512 + c*64 + row;
attention is permutation invariant and the output eviction un-permutes.

Layouts (per partition p):
  qS/kS [128, 16, 1024]  xS[64*jj + d, h, f] = xpT[(2c+j)*64+d, 64h+row]
        (qS only needs the diagonal halves; kS is dup'd via 2 swap DMAs)
  vS    [128, 16, 8, 64] vS[j*64 + krow, h, c, d] = vp[64h+krow, (2c+j)*64+d]
  ET    [128, 8, 2, 512] exp(scores) per head: [key-in-chunk, c, pi, q]
  attn  [128, 8, 1024]   out^T: attn[o%128, o//128, s]

Per head h: scores = 4 quadrant MMs per chunk (row=pi x col=j, K=64,
M=64, N=512) -> exp (ACT, N=1024 ops) -> PV: 2 col-streams x 8
accumulating K=128 MMs -> Z via ones lhsT -> DVE reciprocal + multiply
eviction. The head loop software-pipelines at chunk granularity: the
in-order PE queue gets dense ungated work (PV/Z of the previous head)
before each exp-gated scores matmul, with v-projection and
output-projection tiles interleaved between heads so the PE never
sits idle while ACT computes exp (keeps the HAM clock warm).
"""
import os
import sys

for _p in ("/opt/trn_rl_repo",):
    if os.path.isdir(_p) and _p not in sys.path:
        sys.path.insert(0, _p)

import numpy as np
import ml_dtypes
import concourse.bass as bass
import concourse.mybir as mybir
import concourse.tile as tile
from concourse import bacc
from concourse.bass_utils import run_bass_kernel_spmd

B, S, D, NH, DH = 8, 1024, 1024, 16, 64
P = 128
F32 = mybir.dt.float32
BF16 = mybir.dt.bfloat16
EXP_FN = mybir.ActivationFunctionType.Exp

_CACHE: dict = {}


def _build_nc():
    nc = bacc.Bacc("TRN2", target_bir_lowering=False, debug=False)

    qT = nc.dram_tensor("qT", [D, S], BF16, kind="ExternalInput")
    kT = nc.dram_tensor("kT", [D, S], BF16, kind="ExternalInput")
    vT = nc.dram_tensor("vT", [D, S], BF16, kind="ExternalInput")
    wqT = nc.dram_tensor("wqT", [D, D], BF16, kind="ExternalInput")
    wkT = nc.dram_tensor("wkT", [D, D], BF16, kind="ExternalInput")
    wvT = nc.dram_tensor("wvT", [D, D], BF16, kind="ExternalInput")
    woT = nc.dram_tensor("woT", [D, D], BF16, kind="ExternalInput")
    out = nc.dram_tensor("out", [S, D], F32, kind="ExternalOutput")

    def part3(dram):  # [1024, X] -> [128, 8, X] with row = io*128 + p
        return dram[:].rearrange("(io p) x -> p io x", p=P)

    with tile.TileContext(nc) as tc:
        with tc.tile_pool(name="big", bufs=1) as big, \
             tc.tile_pool(name="pa_x", bufs=2) as pa_x, \
             tc.tile_pool(name="pa_w", bufs=2) as pa_w, \
             tc.tile_pool(name="pet", bufs=2) as pet, \
             tc.tile_pool(name="psb", bufs=2) as psb, \
             tc.tile_pool(name="pconst", bufs=1) as pconst, \
             tc.tile_pool(name="psc", bufs=2, space="PSUM") as psc, \
             tc.tile_pool(name="pnz", bufs=2, space="PSUM") as pnz:

            qS = big.tile([P, NH, S], BF16)
            kS = big.tile([P, NH, S], BF16)
            vS = big.tile([P, NH, 8, DH], BF16)
            attn = big.tile([P, 8, S], BF16)
            out3 = out[:].rearrange("(sc p) o -> p sc o", p=P)

            ones_bf = pconst.tile([P, 64], BF16)
            nc.gpsimd.memset(ones_bf[:], 1.0)
            # warm-up burst: lift the HAM clock gate to 8/8 before the
            # first real matmuls (which are gated on input DMA anyway)
            warm = pconst.tile([P, 128], BF16)
            nc.gpsimd.memset(warm[:], 0.0)
            wps = psc.tile([P, 1024], F32, tag="sc", name="wps")
            for i in range(100):
                nc.tensor.matmul(wps[:, 0:128], warm[:], warm[:],
                                 start=True, stop=True, skip_group_check=True)

            # ---------- input DMAs (sync queue) ----------
            xq = pa_x.tile([P, 8, S], BF16, tag="x")
            wq = pa_w.tile([P, 8, D], BF16, tag="w")
            for io in range(8):  # chunked so first matmuls start early
                nc.sync.dma_start(wq[:, io, :], part3(wqT)[:, io, :])
                nc.sync.dma_start(xq[:, io, :], part3(qT)[:, io, :])
            xk = pa_x.tile([P, 8, S], BF16, tag="x")
            wk = pa_w.tile([P, 8, D], BF16, tag="w")
            for io in range(8):
                nc.sync.dma_start(wk[:, io, :], part3(wkT)[:, io, :])
                nc.sync.dma_start(xk[:, io, :], part3(kT)[:, io, :])

            # ---------- phase A: q/k projections (transposed out) ----------
            def evict_qk(dst, ps_half, mt, nchunk):
                hs0 = nchunk * 8
                nc.vector.tensor_copy(
                    dst[0:64, hs0:hs0 + 8, mt * 64:(mt + 1) * 64],
                    ps_half[0:64].rearrange("p (a b) -> p a b", a=8))
                nc.vector.tensor_copy(
                    dst[64:128, hs0:hs0 + 8, 512 + mt * 64:512 + (mt + 1) * 64],
                    ps_half[64:128].rearrange("p (a b) -> p a b", a=8))

            for nchunk in range(2):
                for xt, wt, dst in ((xq, wq, qS), (xk, wk, kS)):
                    for j in range(4):
                        ps = psc.tile([P, 1024], F32, tag="sc")
                        for f in range(2):
                            mt = 2 * j + f
                            for io in range(8):
                                # two col-split M=64 chains sharing one
                                # rhs stream co-issue on the array
                                for ch in range(2):
                                    nc.tensor.matmul(
                                        ps[ch * 64:(ch + 1) * 64,
                                           f * 512:(f + 1) * 512],
                                        wt[:, io, mt * P + ch * 64:
                                           mt * P + (ch + 1) * 64],
                                        xt[:, io,
                                           nchunk * 512:(nchunk + 1) * 512],
                                        start=(io == 0), stop=(io == 7),
                                        skip_group_check=True)
                        for f in range(2):
                            evict_qk(dst, ps[:, f * 512:(f + 1) * 512],
                                     2 * j + f, nchunk)
                # kS needs both partition-half duplicates
                hs = slice(nchunk * 8, nchunk * 8 + 8)
                nc.gpsimd.dma_start(kS[64:128, hs, 0:512], kS[0:64, hs, 0:512])
                nc.gpsimd.dma_start(kS[0:64, hs, 512:1024],
                                    kS[64:128, hs, 512:1024])

            # ---------- v / wo loads ----------
            xv = pa_x.tile([P, 8, S], BF16, tag="x")
            wv = pa_w.tile([P, 8, D], BF16, tag="w")
            nc.sync.dma_start(wv[:], part3(wvT))
            nc.sync.dma_start(xv[:], part3(vT))
            wo = pa_w.tile([P, 8, D], BF16, tag="w")
            nc.sync.dma_start(wo[:], part3(woT))

            # ---------- helpers emitted inside the head loop ----------
            def evict_vproj(halves, mtv):
                for f in range(2):
                    sc2 = psb.tile([P, 4, 64], BF16, tag="scratch")
                    psn = halves[f]
                    lo = psn[0:64].rearrange("p (a e d) -> p a e d", a=4, e=2)
                    hi = psn[64:128].rearrange("p (a e d) -> p a e d", a=4, e=2)
                    cs = slice(4 * f, 4 * f + 4)
                    nc.vector.tensor_copy(vS[0:64, 2 * mtv, cs, :],
                                          lo[:, :, 0, :])
                    nc.vector.tensor_copy(sc2[0:64], lo[:, :, 1, :])
                    nc.vector.tensor_copy(sc2[64:128], hi[:, :, 0, :])
                    nc.vector.tensor_copy(vS[64:128, 2 * mtv + 1, cs, :],
                                          hi[:, :, 1, :])
                    nc.gpsimd.dma_start(vS[64:128, 2 * mtv, cs, :], sc2[0:64])
                    nc.gpsimd.dma_start(vS[0:64, 2 * mtv + 1, cs, :],
                                        sc2[64:128])

            def emit_vproj(mtv):
                ps = psc.tile([P, 1024], F32, tag="sc")
                for f in range(2):  # nchunk
                    for io in range(8):
                        nc.tensor.matmul(
                            ps[:, f * 512:(f + 1) * 512],
                            xv[:, io, mtv * P:(mtv + 1) * P],
                            wv[:, io, f * 512:(f + 1) * 512],
                            start=(io == 0), stop=(io == 7))
                evict_vproj([ps[:, 0:512], ps[:, 512:1024]], mtv)

            def emit_phaseC(st):
                ps = psc.tile([P, 1024], F32, tag="sc")
                for oc in range(2):
                    for jc in range(8):
                        nc.tensor.matmul(
                            ps[:, oc * 512:(oc + 1) * 512],
                            attn[:, jc, st * P:(st + 1) * P],
                            wo[:, jc, oc * 512:(oc + 1) * 512],
                            start=(jc == 0), stop=(jc == 7))
                res = psb.tile([P, 1024], F32, tag="res")
                nc.vector.tensor_copy(res[:], ps[:])
                nc.gpsimd.dma_start(out3[:, st, :], res[:])

            # ---------- phase B: software-pipelined head loop ----------
            et_tiles = {}
            for h in range(NH + 1):
                ET = None
                if h < NH:
                    ET = pet.tile([P, 8, 2, 512], BF16, tag="et")
                    et_tiles[h] = ET
                hp = h - 1
                ETp = et_tiles.pop(hp) if hp >= 0 else None
                v0 = None
                if h == 0:
                    v0a = pnz.tile([P, 512], F32, tag="num", name="v0a")
                    v0b = pnz.tile([P, 512], F32, tag="z", name="v0b")
                    v0 = [v0a, v0b]
                elif h < NH and h % 2 == 0:
                    emit_vproj(h // 2)
                    if h >= 4:
                        emit_phaseC((h - 4) // 2)
                if ETp is not None:
                    num = pnz.tile([P, 512], F32, tag="num")
                    z = pnz.tile([P, 512], F32, tag="z")
                for r in range(8):
                    if v0 is not None:  # head-0 pipeline fill: vproj(0)
                        for f in range(2):
                            nc.tensor.matmul(
                                v0[f][:], xv[:, r, 0:P],
                                wv[:, r, f * 512:(f + 1) * 512],
                                start=(r == 0), stop=(r == 7),
                                skip_group_check=True)
                    if ETp is not None:
                        for pi in range(2):
                            nc.tensor.matmul(
                                num[64 * pi:64 * pi + 64, :],
                                vS[:, hp, r, :], ETp[:, r, pi, :],
                                start=(r == 0), stop=(r == 7),
                                skip_group_check=True)
                        for pi in range(2):
                            nc.tensor.matmul(
                                z[64 * pi:64 * pi + 64, :],
                                ones_bf[:], ETp[:, r, pi, :],
                                start=(r == 0), stop=(r == 7),
                                skip_group_check=True)
                    if ET is not None:
                        ps = psc.tile([P, 1024], F32, tag="sc")
                        for pi in range(2):
                            row = slice(64 * pi, 64 * pi + 64)
                            rhs = qS[row, h, pi * 512:(pi + 1) * 512]
                            for j in range(2):
                                lhsT = kS[row, h,
                                          j * 512 + r * 64:j * 512 + (r + 1) * 64]
                                nc.tensor.matmul(
                                    ps[64 * j:64 * j + 64,
                                       pi * 512:(pi + 1) * 512], lhsT, rhs,
                                    start=True, stop=True,
                                    skip_group_check=True)
                        nc.scalar.activation(
                            ET[:, r, :, :],
                            ps[:].rearrange("p (a b) -> p a b", a=2), EXP_FN)
                if v0 is not None:
                    evict_vproj([v0[0][:], v0[1][:]], 0)
                if ETp is not None:
                    zr = psb.tile([P, 512], F32, tag="zr", bufs=1)
                    nc.vector.reciprocal_approx_fast(zr[:], z[:])
                    nc.vector.tensor_tensor(
                        attn[:, 0:8, 64 * hp:64 * hp + 64],
                        num[:].rearrange("p (a b) -> p a b", a=8),
                        zr[:].rearrange("p (a b) -> p a b", a=8),
                        mybir.AluOpType.mult)
            emit_phaseC(6)
            emit_phaseC(7)

    nc.compile()
    return nc


def _get_nc():
    if "nc" not in _CACHE:
        _CACHE["nc"] = _build_nc()
    return _CACHE["nc"]


def _prep_inputs(q, k, v, w_q, w_k, w_v, w_o):
    bf = ml_dtypes.bfloat16
    wqT = np.ascontiguousarray((np.asarray(w_q, np.float32) / 8.0).T).astype(bf)
    wkT = np.ascontiguousarray(np.asarray(w_k, np.float32).T).astype(bf)
    wvT = np.ascontiguousarray(np.asarray(w_v, np.float32).T).astype(bf)
    woT = np.ascontiguousarray(np.asarray(w_o, np.float32).T).astype(bf)
    in_maps = []
    for b in range(B):
        in_maps.append({
            "qT": np.ascontiguousarray(np.asarray(q[b], np.float32).T).astype(bf),
            "kT": np.ascontiguousarray(np.asarray(k[b], np.float32).T).astype(bf),
            "vT": np.ascontiguousarray(np.asarray(v[b], np.float32).T).astype(bf),
            "wqT": wqT, "wkT": wkT, "wvT": wvT, "woT": woT,
        })
    return in_maps


def kernel(q, k, v, mask, w_q, w_k, w_v, w_o, **_ignored):
    nc = _get_nc()
    in_maps = _prep_inputs(q, k, v, w_q, w_k, w_v, w_o)
    res = run_bass_kernel_spmd(nc, in_maps, core_ids=list(range(B)))
    return np.stack([res.results[b]["out"] for b in range(B)]).astype(np.float32)
